# revision 16
# baseline (speedup 1.0000x reference)
# Trainium2 Bass kernel for nn_Decoder_73959336837319 (dense_cnn).
#
# Strategy: pure data parallel over batch B=64 -> 8 NeuronCores x 8 batch
# elements. Weights replicated (host pre-transposes them into matmul lhsT
# layout and packs them into a few large contiguous arrays). All matmuls run
# in float32r (TF32-like, full PE rate); everything else fp32.
#
# Per-core program (B_local = 8):
#   P1  transpose inputs[b] (TD,FD)->(FD,TD) via PE transposes
#   P2  encoder pointwise convs w0,w1,w2 (relu,relu,linear)
#   P3  10 encoder highway layers (causal dilated conv k=3 as 3 shifted
#       matmul-accumulates into PSUM; no padding needed: the s=0 tap runs
#       first with start=True over the full time range, shifted taps
#       accumulate into [s:T])
#   P4  attention per batch elem (pos-table gathers via indirect DMA,
#       scores, softmax, ctx, residual) fused with decoder conv ad_w0
#   P5  6 decoder highway layers
#   P6  3 relu convs, final conv, sigmoid outputs (PE-transposed), done-fc
import os
import sys
import math

for _p in ("/opt/trn_rl_repo", "/root/.axon_site/_ro/trn_rl_repo"):
    if os.path.isdir(_p) and _p not in sys.path:
        sys.path.insert(0, _p)

import numpy as np

import concourse.bass as bass
import concourse.mybir as mybir
import concourse.tile as tile
from concourse import bacc
from concourse.bass_utils import run_bass_kernel_spmd
from concourse.masks import make_identity

F32 = mybir.dt.float32
F32R = mybir.dt.float32r
I32 = mybir.dt.int32
AF = mybir.ActivationFunctionType
ALU = mybir.AluOpType
AX = mybir.AxisListType

NCORES = 8
B = 8          # batch elements per core
TE = 500
TD = 400
D = 256
FD = 400
MAXPOS = 512
ENC_DIL = [1, 3, 9, 27, 1, 3, 9, 27, 3, 3]
DEC_DIL = [1, 3, 9, 27, 1, 1]
SQRT_TE = math.sqrt(TE)
SQRT_HALF = math.sqrt(0.5)

# chunk helpers
KC = [(0, 128), (128, 128), (256, 128), (384, 16)]   # 400 into <=128
DC = [(0, 128), (128, 128)]                           # 256 into 128
SC = [(0, 125), (125, 125), (250, 125), (375, 125)]   # 500 into 125


def _wpack_spec():
    """Column layout of the packed pointwise-conv lhsT weights (128 x NW)."""
    spec = {}
    c = 0

    def add(name, tiles):
        nonlocal c
        lst = []
        for (r, w) in tiles:
            lst.append((c, r, w))
            c += w
        spec[name] = lst

    add("w0T", [(r, 256) for (_, r) in KC])
    add("w1T", [(128, 256), (128, 256)])
    add("w2T", [(128, 256), (128, 256)])
    add("wqT", [(128, 256), (128, 256)])
    add("woT", [(128, 256), (128, 256)])
    add("adw0T", [(128, 256)] * 4)
    add("adw1T", [(128, 256), (128, 256)])
    add("adw2T", [(128, 256), (128, 256)])
    add("adw3T", [(128, 256), (128, 256)])
    add("adw4T", [(128, 400), (128, 400)])
    add("fcwT", [(r, 1) for (_, r) in KC])
    return c, spec


def _bpack_spec():
    """Column layout of the packed per-channel biases (128 x NB)."""
    spec = {}
    c = 0

    def add(name, n):
        nonlocal c
        spec[name] = c
        c += n

    add("b0", 2)
    add("b1", 2)
    add("b2", 2)
    add("bq", 2)
    add("bo", 2)
    add("adb0", 2)
    add("adb1", 2)
    add("adb2", 2)
    add("adb3", 2)
    add("adb4", 4)
    add("fcb", 1)
    add("ehwb", 40)   # 10 layers x 4 chunks of 128
    add("dhwb", 24)   # 6 layers x 4 chunks
    return c, spec


DEBUG_DUMPS = False
SYNC_X0 = False
WCOLS, WSPEC = _wpack_spec()
BCOLS, BSPEC = _bpack_spec()


def _emit(tc, T):
    nc = tc.nc

    genp = tc.alloc_tile_pool(name="gen", bufs=1)
    cst = tc.alloc_tile_pool(name="cst", bufs=1)
    hwp = tc.alloc_tile_pool(name="hwp", bufs=2)
    xp = tc.alloc_tile_pool(name="xp", bufs=3)
    awp = tc.alloc_tile_pool(name="awp", bufs=2)
    ps = tc.alloc_tile_pool(name="ps", bufs=8, space="PSUM")

    ident = cst.tile([128, 128], F32, name="ident")
    make_identity(nc, ident[:])

    wpk = cst.tile([128, WCOLS], F32, name="wpk")
    nc.sync.dma_start(wpk[:].bitcast(F32R), T["wpack"][:, :].bitcast(F32R))
    bpk = cst.tile([128, BCOLS], F32, name="bpk")
    nc.sync.dma_start(bpk[:], T["bpack"][:, :])

    def wp(name, ki, m0, msz):
        c0, rows, _ = WSPEC[name][ki]
        return wpk[0:rows, c0 + m0:c0 + m0 + msz].bitcast(F32R)

    def bp(name, chunk, rows=128):
        c = BSPEC[name] + chunk
        return bpk[0:rows, c:c + 1]

    def psum(n=512):
        t = ps.tile([128, 512], F32, tag="ps", name="pst")
        return t

    # persistent ping-pong activation buffers (channel-major, 2 halves of 256)
    gen = {}
    for g in "AB":
        for b in range(B):
            for k in range(2):
                gen[(g, b, k)] = genp.tile(
                    [128, 401], F32, tag=f"g{g}{b}{k}", name=f"g{g}{b}{k}")
                nc.gpsimd.memset(gen[(g, b, k)][:, 0:1], 0.0)

    # ---------------- P1+P2: transpose inputs, encoder pointwise convs ----
    for b in range(B):
        xin = []
        for ti, (t0, tsz) in enumerate(KC):
            xi = xp.tile([128, 400], F32, tag="xin", bufs=5, name=f"xin{b}_{ti}")
            nc.sync.dma_start(xi[:tsz, :], T["inputs"][b, t0:t0 + tsz, :])
            xin.append((xi, t0, tsz))
        x0 = []
        for fi, (f0, fsz) in enumerate(KC):
            x0t = xp.tile([128, 400], F32, tag=f"x0{fi}", bufs=2,
                          name=f"x0_{b}_{fi}")
            for (xi, t0, tsz) in xin:
                pt = psum()
                nc.tensor.transpose(pt[:fsz, :tsz], in_=xi[:tsz, f0:f0 + fsz],
                                    identity=ident[:tsz, :tsz])
                nc.vector.tensor_copy(x0t[:fsz, t0:t0 + tsz].bitcast(F32R),
                                      pt[:fsz, :tsz])
            x0.append((x0t, fsz))
            if True:
                nc.sync.dma_start(T["dbg_x0"][b, fi], x0t[:, :])
        x1 = []
        for mi, (m0, msz) in enumerate(DC):
            pt = psum()
            for ki, (x0t, ksz) in enumerate(x0):
                nc.tensor.matmul(pt[:128, :400], lhsT=wp("w0T", ki, m0, 128),
                                 rhs=x0t[:ksz, :].bitcast(F32R),
                                 start=(ki == 0), stop=(ki == 3))
            x1t = xp.tile([128, 400], F32, tag=f"x1{mi}", bufs=1, name=f"x1_{b}_{mi}")
            nc.scalar.activation(x1t[:].bitcast(F32R), pt[:128, :400],
                                 AF.Relu, bias=bp("b0", mi))
            x1.append(x1t)
        x2 = []
        for mi, (m0, msz) in enumerate(DC):
            pt = psum()
            for ki in range(2):
                nc.tensor.matmul(pt[:128, :400], lhsT=wp("w1T", ki, m0, 128),
                                 rhs=x1[ki][:].bitcast(F32R),
                                 start=(ki == 0), stop=(ki == 1))
            x2t = xp.tile([128, 400], F32, tag=f"x2{mi}", bufs=1, name=f"x2_{b}_{mi}")
            nc.scalar.activation(x2t[:].bitcast(F32R), pt[:128, :400],
                                 AF.Relu, bias=bp("b1", mi))
            x2.append(x2t)
        for mi, (m0, msz) in enumerate(DC):
            pt = psum()
            for ki in range(2):
                nc.tensor.matmul(pt[:128, :400], lhsT=wp("w2T", ki, m0, 128),
                                 rhs=x2[ki][:].bitcast(F32R),
                                 start=(ki == 0), stop=(ki == 1))
            nc.scalar.activation(gen[("B", b, mi)][:, 1:401].bitcast(F32R),
                                 pt[:128, :400], AF.Identity, bias=bp("b2", mi))
            if True:
                nc.sync.dma_start(T["dbg_enc"][0, b, mi], gen[("B", b, mi)][:, 1:401])

    # ---------------- highway layer ----------------
    def highway(lidx, dil, src, dst, bname, bchunk0):
        hw = hwp.tile([128, 3072], F32, tag="hw", name=f"hw{lidx}")
        nc.sync.dma_start(hw[:].bitcast(F32R), T["hwpack"][lidx].bitcast(F32R))
        taps = [(2, 0), (1, dil), (0, 2 * dil)]
        for b in range(B):
            P = []
            for mi in range(4):
                pt = psum()
                first = True
                for (j, s) in taps:
                    for k in range(2):
                        lhs = hw[:, (j * 2 + k) * 512 + mi * 128:
                                 (j * 2 + k) * 512 + mi * 128 + 128].bitcast(F32R)
                        src_t = gen[(src, b, k)]
                        if s % 2 == 0:
                            # dst offset s (even), N = 400-s (even)
                            nc.tensor.matmul(pt[:128, s:400], lhsT=lhs,
                                             rhs=src_t[:, 1:401 - s].bitcast(F32R),
                                             start=first, stop=(j == 0 and k == 1))
                        else:
                            # odd shift: start one col early, reading the
                            # leading zero column -> dst offset s-1 (even),
                            # N = 401-s (even)
                            nc.tensor.matmul(pt[:128, s - 1:400], lhsT=lhs,
                                             rhs=src_t[:, 0:401 - s].bitcast(F32R),
                                             start=False, stop=(j == 0 and k == 1))
                        first = False
                P.append(pt)
            for k in range(2):
                tt = awp.tile([128, 400], F32, tag="hwt", bufs=2,
                              name=f"tt{lidx}_{b}_{k}")
                nc.scalar.activation(tt[:], P[k][:128, :400], AF.Sigmoid,
                                     bias=bp(bname, bchunk0 + k))
                x_old = gen[(src, b, k)][:, 1:401]
                d = awp.tile([128, 400], F32, tag="hws", bufs=3,
                             name=f"d{lidx}_{b}_{k}")
                nc.vector.scalar_tensor_tensor(
                    out=d[:], in0=P[2 + k][:128, :400],
                    scalar=bp(bname, bchunk0 + 2 + k), in1=x_old[:],
                    op0=ALU.add, op1=ALU.subtract)
                m = awp.tile([128, 400], F32, tag="hws", bufs=3,
                             name=f"m{lidx}_{b}_{k}")
                nc.vector.tensor_mul(m[:], tt[:], d[:])
                nc.vector.tensor_add(gen[(dst, b, k)][:, 1:401].bitcast(F32R),
                                     m[:], x_old[:])
                if lidx < 11:
                    nc.sync.dma_start(T["dbg_enc"][1 + lidx, b, k],
                                      gen[(dst, b, k)][:, 1:401])

    # ---------------- P3: encoder highways ----------------
    cur, nxt = "B", "A"
    for i, dil in enumerate(ENC_DIL):
        highway(i, dil, cur, nxt, "ehwb", 4 * i)
        cur, nxt = nxt, cur
    assert cur == "B"  # encoder output lives in gen B

    # ---------------- P4: attention + ad_w0 ----------------
    for b in range(B):
        vv = []
        for si, (s0, ssz) in enumerate(SC):
            v = awp.tile([128, 256], F32, tag=f"vv{si}", bufs=2, name=f"vv{b}_{si}")
            nc.sync.dma_start(v[:ssz, :].bitcast(F32R),
                              T["values"][b, s0:s0 + ssz, :].bitcast(F32R))
            vv.append(v)
        # keys + positional gather, added, then transposed to (256, 500)
        kpeT = [awp.tile([128, 500], F32, tag=f"kpeT{h}", bufs=1, name=f"kpeT{b}_{h}")
                for h in range(2)]
        for si, (s0, ssz) in enumerate(SC):
            kt = awp.tile([128, 256], F32, tag="s256", bufs=3, name=f"kt{b}_{si}")
            nc.sync.dma_start(kt[:ssz, :], T["keys"][b, s0:s0 + ssz, :])
            ki_t = awp.tile([128, 1], I32, tag=f"ki{si}", name=f"ki{b}_{si}")
            nc.sync.dma_start(ki_t[:ssz, :], T["tpos"][b, s0:s0 + ssz, :])
            kg = awp.tile([128, 256], F32, tag="s256", bufs=3, name=f"kg{b}_{si}")
            nc.gpsimd.indirect_dma_start(
                out=kg[:ssz, :], out_offset=None, in_=T["ktab"][:, :],
                in_offset=bass.IndirectOffsetOnAxis(ap=ki_t[:ssz, :1], axis=0))
            kp = awp.tile([128, 256], F32, tag=f"kp{si}", bufs=1, name=f"kp{b}_{si}")
            nc.vector.tensor_add(kp[:ssz, :], kt[:ssz, :], kg[:ssz, :])
            for h in range(2):
                pt = psum()
                nc.tensor.transpose(pt[:128, :ssz],
                                    in_=kp[:ssz, h * 128:(h + 1) * 128],
                                    identity=ident[:ssz, :ssz])
                nc.vector.tensor_copy(kpeT[h][:, s0:s0 + ssz].bitcast(F32R),
                                      pt[:128, :ssz])
        # frame-position gather -> fposT (256, 400)
        fposT = [awp.tile([128, 400], F32, tag=f"fposT{h}", bufs=1, name=f"fposT{b}_{h}")
                 for h in range(2)]
        for ti, (t0, tsz) in enumerate(KC):
            fi_t = awp.tile([128, 1], I32, tag=f"fi{ti}", name=f"fi{b}_{ti}")
            nc.sync.dma_start(fi_t[:tsz, :], T["fpos"][b, t0:t0 + tsz, :])
            fg = awp.tile([128, 256], F32, tag="s256", bufs=3, name=f"fg{b}_{ti}")
            nc.gpsimd.indirect_dma_start(
                out=fg[:tsz, :], out_offset=None, in_=T["qtab"][:, :],
                in_offset=bass.IndirectOffsetOnAxis(ap=fi_t[:tsz, :1], axis=0))
            for h in range(2):
                pt = psum()
                nc.tensor.transpose(pt[:128, :tsz],
                                    in_=fg[:tsz, h * 128:(h + 1) * 128],
                                    identity=ident[:tsz, :tsz])
                nc.vector.tensor_copy(fposT[h][:, t0:t0 + tsz], pt[:128, :tsz])
        if True:
            for h in range(2):
                nc.sync.dma_start(T["dbg_kpeT"][b, h], kpeT[h][:, :])
                nc.sync.dma_start(T["dbg_fpT"][b, h], fposT[h][:, :])
        # qT = x_enc + fposT
        qT = [awp.tile([128, 400], F32, tag=f"qT{h}", bufs=1, name=f"qT{b}_{h}")
              for h in range(2)]
        for h in range(2):
            nc.vector.tensor_add(qT[h][:].bitcast(F32R), gen[("B", b, h)][:, 1:401],
                                 fposT[h][:])
        if True:
            for h in range(2):
                nc.sync.dma_start(T["dbg_qT"][b, h], qT[h][:, :])
        # pT = wq @ qT + bq
        pT = [awp.tile([128, 400], F32, tag=f"pT{h}", bufs=1, name=f"pT{b}_{h}")
              for h in range(2)]
        for h, (m0, msz) in enumerate(DC):
            pt = psum()
            for ki in range(2):
                nc.tensor.matmul(pt[:128, :400], lhsT=wp("wqT", ki, m0, 128),
                                 rhs=qT[ki][:].bitcast(F32R),
                                 start=(ki == 0), stop=(ki == 1))
            nc.scalar.activation(pT[h][:].bitcast(F32R), pt[:128, :400],
                                 AF.Identity, bias=bp("bq", h))
        if True:
            for h in range(2):
                nc.sync.dma_start(T["dbg_pT"][b, h], pT[h][:, :])
        # scores + softmax, row chunks of TD
        attn_sb = []
        for mi, (m0, msz) in enumerate(KC):
            pt = psum()
            for ki in range(2):
                nc.tensor.matmul(pt[:msz, :500],
                                 lhsT=pT[ki][:, m0:m0 + msz].bitcast(F32R),
                                 rhs=kpeT[ki][:].bitcast(F32R),
                                 start=(ki == 0), stop=(ki == 1))
            negmax = awp.tile([128, 1], F32, tag="s1", bufs=8,
                              name=f"negmax{b}_{mi}")
            nc.vector.tensor_reduce(negmax[:msz, :], pt[:msz, :500],
                                    axis=AX.X, op=ALU.max, negate=True)
            esum = awp.tile([128, 1], F32, tag="s1", bufs=8,
                            name=f"esum{b}_{mi}")
            at = awp.tile([128, 500], F32, tag="at", bufs=4, name=f"at{b}_{mi}")
            nc.scalar.activation(at[:msz, :], pt[:msz, :500], AF.Exp,
                                 bias=negmax[:msz, :1], scale=1.0,
                                 accum_out=esum[:msz, :1])
            rsum = awp.tile([128, 1], F32, tag="s1", bufs=8,
                            name=f"rsum{b}_{mi}")
            nc.vector.reciprocal(rsum[:msz, :], esum[:msz, :])
            nc.vector.tensor_scalar_mul(at[:msz, :], at[:msz, :],
                                         rsum[:msz, :1])
            nc.sync.dma_start(T["out_attn"][b, m0:m0 + msz, :], at[:msz, :])
            attn_sb.append(at)
        # attn^T scaled by sqrt(TE)
        atT = [awp.tile([128, 400], F32, tag=f"atT{si}", bufs=2, name=f"atT{b}_{si}")
               for si in range(4)]
        for si, (s0, ssz) in enumerate(SC):
            for mi, (m0, msz) in enumerate(KC):
                pt = psum()
                nc.tensor.transpose(pt[:ssz, :msz],
                                    in_=attn_sb[mi][:msz, s0:s0 + ssz],
                                    identity=ident[:msz, :msz])
                nc.vector.tensor_scalar_mul(
                    atT[si][:ssz, m0:m0 + msz].bitcast(F32R),
                    pt[:ssz, :msz], SQRT_TE)
        # ctxT = values^T @ attn^T
        ctxT = [awp.tile([128, 400], F32, tag=f"ctxT{h}", bufs=1, name=f"ctxT{b}_{h}")
                for h in range(2)]
        for h in range(2):
            pt = psum()
            for si, (s0, ssz) in enumerate(SC):
                nc.tensor.matmul(pt[:128, :400],
                                 lhsT=vv[si][:ssz, h * 128:(h + 1) * 128].bitcast(F32R),
                                 rhs=atT[si][:ssz, :].bitcast(F32R),
                                 start=(si == 0), stop=(si == 3))
            nc.vector.tensor_copy(ctxT[h][:].bitcast(F32R), pt[:128, :400])
        if True:
            for h in range(2):
                nc.sync.dma_start(T["dbg_ctxT"][b, h], ctxT[h][:, :])
        # RT = (wo_s @ ctxT + bo_s) + sqrt(.5) * qT   (wo/bo pre-scaled on host)
        RT = [awp.tile([128, 400], F32, tag=f"RT{h}", bufs=1, name=f"RT{b}_{h}")
              for h in range(2)]
        for h, (m0, msz) in enumerate(DC):
            pt = psum()
            for ki in range(2):
                nc.tensor.matmul(pt[:128, :400], lhsT=wp("woT", ki, m0, 128),
                                 rhs=ctxT[ki][:].bitcast(F32R),
                                 start=(ki == 0), stop=(ki == 1))
            rtmp = awp.tile([128, 400], F32, tag="hws", bufs=3, name=f"rtmp{b}_{h}")
            nc.scalar.activation(rtmp[:], pt[:128, :400], AF.Identity,
                                 bias=bp("bo", h))
            nc.vector.scalar_tensor_tensor(
                out=RT[h][:].bitcast(F32R), in0=qT[h][:], scalar=SQRT_HALF,
                in1=rtmp[:], op0=ALU.mult, op1=ALU.add)
        if True:
            for h in range(2):
                nc.sync.dma_start(T["dbg_RT"][b, h], RT[h][:, :])
        # ad_w0 over concat([RT, Q]) -> gen A
        rhs_list = [RT[0][:, 0:400], RT[1][:, 0:400],
                    gen[("B", b, 0)][:, 1:401], gen[("B", b, 1)][:, 1:401]]
        for h, (m0, msz) in enumerate(DC):
            pt = psum()
            for ki in range(4):
                nc.tensor.matmul(pt[:128, :400], lhsT=wp("adw0T", ki, m0, 128),
                                 rhs=rhs_list[ki].bitcast(F32R),
                                 start=(ki == 0), stop=(ki == 3))
            nc.scalar.activation(gen[("A", b, h)][:, 1:401].bitcast(F32R),
                                 pt[:128, :400], AF.Identity, bias=bp("adb0", h))
            if True:
                nc.sync.dma_start(T["dbg_xd0"][b, h], gen[("A", b, h)][:, 1:401])

    # ---------------- P5: decoder highways ----------------
    cur, nxt = "A", "B"
    for i, dil in enumerate(DEC_DIL):
        highway(10 + i, dil, cur, nxt, "dhwb", 4 * i)
        cur, nxt = nxt, cur
    assert cur == "A"  # decoder output lives in gen A

    # ---------------- P6: decoder tail ----------------
    for b in range(B):
        curt = [gen[("A", b, 0)][:, 1:401], gen[("A", b, 1)][:, 1:401]]
        for li, (wname, bname) in enumerate(
                (("adw1T", "adb1"), ("adw2T", "adb2"), ("adw3T", "adb3"))):
            nxtt = []
            for h, (m0, msz) in enumerate(DC):
                pt = psum()
                for ki in range(2):
                    nc.tensor.matmul(pt[:128, :400],
                                     lhsT=wp(wname, ki, m0, 128),
                                     rhs=curt[ki].bitcast(F32R),
                                     start=(ki == 0), stop=(ki == 1))
                nt = awp.tile([128, 400], F32, tag="dx", bufs=4,
                              name=f"dx{b}_{li}_{h}")
                nc.scalar.activation(nt[:].bitcast(F32R), pt[:128, :400],
                                     AF.Relu, bias=bp(bname, h))
                nxtt.append(nt)
            curt = nxtt
        # y = adw4 @ x + adb4  (FD-major, pre-sigmoid)
        ysb = []
        for mi, (m0, msz) in enumerate(KC):
            pt = psum()
            for ki in range(2):
                nc.tensor.matmul(pt[:msz, :400], lhsT=wp("adw4T", ki, m0, msz),
                                 rhs=curt[ki].bitcast(F32R),
                                 start=(ki == 0), stop=(ki == 1))
            yt = awp.tile([128, 400], F32, tag=f"y{mi}", bufs=1, name=f"y{b}_{mi}")
            nc.scalar.activation(yt[:msz, :].bitcast(F32R), pt[:msz, :400],
                                 AF.Identity, bias=bp("adb4", mi, rows=msz))
            ysb.append(yt)
        # done = sigmoid(fc_w . y + fc_b)
        pt = psum()
        for ki, (k0, ksz) in enumerate(KC):
            nc.tensor.matmul(pt[:1, :400], lhsT=wp("fcwT", ki, 0, 1),
                             rhs=ysb[ki][:ksz, :].bitcast(F32R),
                             start=(ki == 0), stop=(ki == 3))
        dsb = awp.tile([128, 400], F32, tag="yTs", bufs=2, name=f"dsb{b}")
        nc.scalar.activation(dsb[:1, :], pt[:1, :400], AF.Sigmoid,
                             bias=bp("fcb", 0, rows=1))
        nc.sync.dma_start(T["out_done"][b, :, :], dsb[:1, :])
        # outputs = sigmoid(y)^T
        for ti, (t0, tsz) in enumerate(KC):
            yT = awp.tile([128, 400], F32, tag="yTs", bufs=2, name=f"yT{b}_{ti}")
            for mi, (m0, msz) in enumerate(KC):
                pt = psum()
                nc.tensor.transpose(pt[:tsz, :msz],
                                    in_=ysb[mi][:msz, t0:t0 + tsz],
                                    identity=ident[:msz, :msz])
                nc.scalar.activation(yT[:tsz, m0:m0 + msz], pt[:tsz, :msz],
                                     AF.Sigmoid)
            nc.sync.dma_start(T["out_main"][b, t0:t0 + tsz, :], yT[:tsz, :])

    for p in (ps, awp, xp, hwp, cst, genp):
        p.release()


def _build_nc():
    nc = bacc.Bacc("TRN2", target_bir_lowering=False, debug=False,
                   enable_asserts=False, num_devices=NCORES)
    T = {}
    T["keys"] = nc.dram_tensor("keys", (B, TE, D), F32, kind="ExternalInput").ap()
    T["values"] = nc.dram_tensor("values", (B, TE, D), F32, kind="ExternalInput").ap()
    T["inputs"] = nc.dram_tensor("inputs", (B, TD, FD), F32, kind="ExternalInput").ap()
    T["tpos"] = nc.dram_tensor("tpos", (B, TE, 1), I32, kind="ExternalInput").ap()
    T["fpos"] = nc.dram_tensor("fpos", (B, TD, 1), I32, kind="ExternalInput").ap()
    T["ktab"] = nc.dram_tensor("ktab", (MAXPOS, D), F32, kind="ExternalInput").ap()
    T["qtab"] = nc.dram_tensor("qtab", (MAXPOS, D), F32, kind="ExternalInput").ap()
    T["wpack"] = nc.dram_tensor("wpack", (128, WCOLS), F32, kind="ExternalInput").ap()
    T["bpack"] = nc.dram_tensor("bpack", (128, BCOLS), F32, kind="ExternalInput").ap()
    T["hwpack"] = nc.dram_tensor("hwpack", (16, 128, 3072), F32, kind="ExternalInput").ap()
    T["out_main"] = nc.dram_tensor("out_main", (B, TD, FD), F32, kind="ExternalOutput").ap()
    DK = "ExternalOutput" if DEBUG_DUMPS else "Internal"
    if True:
        T["dbg_x0"] = nc.dram_tensor("dbg_x0", (B, 4, 128, 400), F32, kind=DK).ap()
        T["dbg_enc"] = nc.dram_tensor("dbg_enc", (12, B, 2, 128, 400), F32, kind=DK).ap()
        T["dbg_qT"] = nc.dram_tensor("dbg_qT", (B, 2, 128, 400), F32, kind=DK).ap()
        T["dbg_pT"] = nc.dram_tensor("dbg_pT", (B, 2, 128, 400), F32, kind=DK).ap()
        T["dbg_kpeT"] = nc.dram_tensor("dbg_kpeT", (B, 2, 128, 500), F32, kind=DK).ap()
        T["dbg_ctxT"] = nc.dram_tensor("dbg_ctxT", (B, 2, 128, 400), F32, kind=DK).ap()
        T["dbg_RT"] = nc.dram_tensor("dbg_RT", (B, 2, 128, 400), F32, kind=DK).ap()
        T["dbg_xd0"] = nc.dram_tensor("dbg_xd0", (B, 2, 128, 400), F32, kind=DK).ap()
        T["dbg_fpT"] = nc.dram_tensor("dbg_fpT", (B, 2, 128, 400), F32, kind=DK).ap()
    T["out_attn"] = nc.dram_tensor("out_attn", (B, TD, TE), F32, kind="ExternalOutput").ap()
    T["out_done"] = nc.dram_tensor("out_done", (B, TD, 1), F32, kind="ExternalOutput").ap()

    with tile.TileContext(nc) as tc:
        _emit(tc, T)
    nc.compile()
    return nc


def _pack_weights(inp):
    wpack = np.zeros((128, WCOLS), np.float32)

    def fillw(name, WT):
        k0 = 0
        for (c0, rows, cols) in WSPEC[name]:
            wpack[:rows, c0:c0 + cols] = WT[k0:k0 + rows, :]
            k0 += rows

    fillw("w0T", np.ascontiguousarray(inp["ae_w0"][:, :, 0].T))
    fillw("w1T", np.ascontiguousarray(inp["ae_w1"][:, :, 0].T))
    fillw("w2T", np.ascontiguousarray(inp["ae_w2"][:, :, 0].T))
    fillw("wqT", np.ascontiguousarray(inp["attn_wq"].T))
    fillw("woT", np.ascontiguousarray(inp["attn_wo"].T) * np.float32(SQRT_HALF))
    fillw("adw0T", np.ascontiguousarray(inp["ad_w0"][:, :, 0].T))
    fillw("adw1T", np.ascontiguousarray(inp["ad_w1"][:, :, 0].T))
    fillw("adw2T", np.ascontiguousarray(inp["ad_w2"][:, :, 0].T))
    fillw("adw3T", np.ascontiguousarray(inp["ad_w3"][:, :, 0].T))
    fillw("adw4T", np.ascontiguousarray(inp["ad_w4"][:, :, 0].T))
    fillw("fcwT", np.ascontiguousarray(inp["fc_w"].T))

    bpack = np.zeros((128, BCOLS), np.float32)

    def fillb(name, vec, chunk0=0):
        c = BSPEC[name] + chunk0
        n = vec.shape[0]
        for i in range((n + 127) // 128):
            r = min(128, n - i * 128)
            bpack[:r, c + i] = vec[i * 128:i * 128 + r]

    fillb("b0", inp["ae_b0"])
    fillb("b1", inp["ae_b1"])
    fillb("b2", inp["ae_b2"])
    fillb("bq", inp["attn_bq"])
    fillb("bo", inp["attn_bo"] * np.float32(SQRT_HALF))
    fillb("adb0", inp["ad_b0"])
    fillb("adb1", inp["ad_b1"])
    fillb("adb2", inp["ad_b2"])
    fillb("adb3", inp["ad_b3"])
    fillb("adb4", inp["ad_b4"])
    fillb("fcb", inp["fc_b"])
    for i in range(10):
        fillb("ehwb", inp["ae_hw_b"][i], 4 * i)
    for i in range(6):
        fillb("dhwb", inp["ad_hw_b"][i], 4 * i)

    def hwtr(w):
        L = w.shape[0]
        t = np.ascontiguousarray(np.transpose(w, (0, 3, 2, 1)))   # (L,3,256,512)
        t = t.reshape(L, 3, 2, 128, 512).transpose(0, 3, 1, 2, 4)
        return np.ascontiguousarray(t.reshape(L, 128, 3072))

    hwpack = np.concatenate([hwtr(inp["ae_hw_w"]), hwtr(inp["ad_hw_w"])], axis=0)
    return wpack, bpack, np.ascontiguousarray(hwpack)


_NC_CACHE = {}


def _get_nc():
    if "nc" not in _NC_CACHE:
        _NC_CACHE["nc"] = _build_nc()
    return _NC_CACHE["nc"]


def _run(inputs, trace=False):
    nc = _get_nc()
    inp = {k: np.asarray(v) for k, v in inputs.items()}
    wpack, bpack, hwpack = _pack_weights(inp)
    in_maps = []
    for c in range(NCORES):
        s = slice(c * B, (c + 1) * B)
        in_maps.append({
            "keys": np.ascontiguousarray(inp["keys"][s]),
            "values": np.ascontiguousarray(inp["values"][s]),
            "inputs": np.ascontiguousarray(inp["inputs"][s]),
            "tpos": np.ascontiguousarray(inp["text_positions"][s].astype(np.int32)[..., None]),
            "fpos": np.ascontiguousarray(inp["frame_positions"][s].astype(np.int32)[..., None]),
            "ktab": inp["key_pos_table"],
            "qtab": inp["query_pos_table"],
            "wpack": wpack,
            "bpack": bpack,
            "hwpack": hwpack,
        })
    res = run_bass_kernel_spmd(nc, in_maps, list(range(NCORES)))
    outputs = np.concatenate([res.results[c]["out_main"] for c in range(NCORES)], axis=0)
    attn = np.concatenate([res.results[c]["out_attn"] for c in range(NCORES)], axis=0)
    done = np.concatenate([res.results[c]["out_done"] for c in range(NCORES)], axis=0)
    return (outputs, attn[None], done), res


def kernel(**inputs):
    out, _ = _run(inputs, trace=False)
    return out
